# revision 1
# baseline (speedup 1.0000x reference)
"""Causal GQA attention (B=4, S=2048, H=16, HK=4, D=128) on 8 trn2 cores.

Sharding: 16 (request, kv-head) units, 2 per core. Each unit owns 4 query
heads that share one K/V head (GQA group).

Per-core kernel (per head):
  - scores are computed TRANSPOSED: S_T[sk, sq] = K_chunk^T-stationary
    matmul with Q^T moving (f32r, free dim 256 => full PE rate), so no
    P-transposes are ever needed.
  - exp on ScalarE straight out of PSUM with the 1/sqrt(D) scale folded in,
    output bf16 to SBUF.
  - causal masking is multiplicative on the two diagonal chunks per group.
  - PV: lhsT = expT chunk [sk,128sq] (bf16 stationary), rhs = V' chunk
    [sk, 129] where column 128 is ones => row-sums accumulate for free in
    PSUM column 128, and the output lands already [sq, d].
  - normalize with DVE reciprocal + tensor_scalar_mul, DMA out.
"""

import sys

if "/opt/trn_rl_repo" not in sys.path:
    sys.path.insert(0, "/opt/trn_rl_repo")

from contextlib import ExitStack

import ml_dtypes
import numpy as np

import concourse.bass as bass  # noqa: F401  (registers engine classes)
import concourse.tile as tile
from concourse import bacc, mybir
from concourse.bass_utils import run_bass_kernel_spmd

B = 4
S = 2048
H = 16
HK = 4
D = 128
REP = H // HK          # query heads per kv head
SCALE = float(1.0 / np.float32(np.sqrt(D)))

NCORES = 8
NU = 2                 # units (request, kv-head) per core
NHL = REP              # query heads per unit
P = 128
NT = S // P            # 16 sk/sq tiles per sequence
SQG = 256              # sq columns per group (>=256 keeps f32r at full rate)
NG = S // SQG          # 8 groups per head
GB = 6                 # score chunks per PSUM batch ([128, 6*256] = 3 banks)

F32 = mybir.dt.float32
F32R = mybir.dt.float32r
BF16 = mybir.dt.bfloat16

_CACHE = {}


def _build_program(qk_f32r: bool = True, pipe: int = 2, split_loads: bool = False):
    nc = bacc.Bacc("TRN2", target_bir_lowering=False, debug=False,
                   num_devices=NCORES)
    QKDT = F32R if qk_f32r else F32
    qT = nc.dram_tensor("qT", [NU, NHL, P, S], QKDT, kind="ExternalInput").ap()
    kT = nc.dram_tensor("kT", [NU, P, S], QKDT, kind="ExternalInput").ap()
    vp = nc.dram_tensor("vp", [NU, P, NT, D + 1], BF16, kind="ExternalInput").ap()
    masks = nc.dram_tensor("masks", [P, P], BF16, kind="ExternalInput").ap()
    # [unit, head, sq_partition, q_tile, d] -- one DMA per head, 8KB rows
    out = nc.dram_tensor("out", [NU, NHL, P, NT, D], F32, kind="ExternalOutput").ap()

    with tile.TileContext(nc) as tc, ExitStack() as ctx:
        kpool = ctx.enter_context(tc.tile_pool(name="kT", bufs=2))
        qpool = ctx.enter_context(tc.tile_pool(name="qT", bufs=2))
        vpool = ctx.enter_context(tc.tile_pool(name="vp", bufs=2))
        epool = ctx.enter_context(tc.tile_pool(name="expT", bufs=3))
        mpool = ctx.enter_context(tc.tile_pool(name="masks", bufs=1))
        opool = ctx.enter_context(tc.tile_pool(name="osb", bufs=2))
        rpool = ctx.enter_context(tc.tile_pool(name="recip", bufs=4))
        spsum = ctx.enter_context(tc.tile_pool(name="scores", bufs=2, space="PSUM"))
        ppsum = ctx.enter_context(tc.tile_pool(name="pv", bufs=2, space="PSUM"))

        mtile = mpool.tile([P, P], BF16)

        # PE HAM warm-up: ~3.5us of dummy matmuls on scratch SBUF during the
        # initial K/Q DMA wait, so the first real QK batch runs at full clock.
        wpool = ctx.enter_context(tc.tile_pool(name="warm", bufs=1))
        wtile = wpool.tile([P, 512], F32R)
        nc.vector.memset(wtile[:].bitcast(mybir.dt.uint32), 0)
        wpsum = spsum.tile([P, GB * SQG], F32, tag="scores")
        for _ in range(8):
            nc.tensor.matmul(wpsum[:, :512], lhsT=wtile[:, :P],
                             rhs=wtile[:], start=True, stop=True)

        HEADCOLS = sum((2 * g + 1) * SQG + P for g in range(NG))  # 17408
        BCOLS = GB * SQG                 # psum batch capacity (3 banks)

        def emit_pv(vtile, etile, colof, osb, g):
            """PV + normalize for the two q-tiles of group g."""
            for cc in range(2):
                t = 2 * g + cc
                ptile = ppsum.tile([P, D + 1], F32, tag="pv")
                for j in range(t + 1):
                    c0 = colof[(g, j)] + (0 if (cc == 1 and j == t) else cc * P)
                    nc.tensor.matmul(
                        ptile[:],
                        lhsT=etile[:, c0:c0 + P],
                        rhs=vtile[:, j, :],
                        start=(j == 0), stop=(j == t),
                    )
                rec = rpool.tile([P, 1], F32)
                nc.vector.reciprocal(rec[:], ptile[:, D:D + 1])
                nc.vector.tensor_scalar_mul(osb[:, t, :], ptile[:, 0:D], rec[:])

        # Software pipeline: PV of group g-1/g-2 is emitted between the
        # QK/exp batches of later groups so ScalarE (the bottleneck) never
        # waits on PE, and PE fills its slack with PV.
        pend = []         # [(vtile, etile, colof, osb, g, store)]
        PIPE = pipe

        def flush_pend():
            v, e, cof, o, g, store = pend.pop(0)
            emit_pv(v, e, cof, o, g)
            if store is not None:
                for eng, dst, src in store:
                    eng.dma_start(dst, src)

        for u in range(NU):
            ktile = kpool.tile([P, S], QKDT)
            if u == 0:
                # first batch needs only k chunks 0..5: land them early on SP
                # and push the remainder to the idle ACT queue
                nc.sync.dma_start(ktile[:, :768], kT[u][:, :768])
                nc.scalar.dma_start(ktile[:, 768:], kT[u][:, 768:])
            else:
                nc.sync.dma_start(ktile[:], kT[u])
            vtile = vpool.tile([P, NT, D + 1], BF16)
            for hl in range(NHL):
                qtile = qpool.tile([P, S], QKDT)
                if u == 0 and hl == 0:
                    # groups run big->small, so the first pair only reads
                    # q columns [1536:2048]: land those first
                    nc.gpsimd.dma_start(qtile[:, 1536:], qT[u, hl][:, 1536:])
                    nc.gpsimd.dma_start(qtile[:, :1536], qT[u, hl][:, :1536])
                else:
                    nc.sync.dma_start(qtile[:], qT[u, hl])
                if u == 0 and hl == 0:
                    nc.gpsimd.dma_start(mtile[:], masks[:])
                if hl == 0:
                    nc.gpsimd.dma_start(vtile[:], vp[u])
                osb = opool.tile([P, NT, D], F32)
                etile = epool.tile([P, HEADCOLS], BF16)
                last_head = (u == NU - 1 and hl == NHL - 1)
                # groups big->small: every head ends on tiny PV work and the
                # next head opens with big QK batches, keeping ScalarE fed
                # across head boundaries (and the kernel tail short)
                gs = list(range(NG - 1, -1, -1))

                # chunk stream for this head: full 256-wide chunks of a PAIR
                # of groups, then their two 128-wide diagonal half-chunks
                # back-to-back. Keeps every 256-wide PSUM write 256-aligned so
                # no matmul output crosses a PSUM bank boundary.
                chunks = []
                for ga, gb in zip(gs[0::2], gs[1::2]):
                    for j in range(2 * ga + 1):
                        chunks.append((ga, j, SQG, ga * SQG))
                    chunks.append((ga, 2 * ga + 1, P, ga * SQG + P))
                    chunks.append((gb, 2 * gb + 1, P, gb * SQG + P))
                    for j in range(2 * gb + 1):
                        chunks.append((gb, j, SQG, gb * SQG))
                remaining = {g: 2 * g + 2 for g in gs}
                colof = {}
                acc = 0
                for (g, j, w, qc) in chunks:
                    colof[(g, j)] = acc
                    acc += w

                def do_batch(batch, bcols):
                    stile = spsum.tile([P, BCOLS], F32, tag="scores")
                    ncols = 0
                    for (g, j, w, qc) in batch:
                        nc.tensor.matmul(
                            stile[:, ncols:ncols + w],
                            lhsT=ktile[:, j * P:(j + 1) * P],
                            rhs=qtile[:, qc:qc + w],
                            start=True, stop=True,
                        )
                        ncols += w
                    e0 = colof[(batch[0][0], batch[0][1])]
                    nc.scalar.activation(
                        etile[:, e0:e0 + ncols],
                        stile[:, :ncols],
                        mybir.ActivationFunctionType.Exp,
                        scale=SCALE,
                    )
                    # a group is complete once all its chunks are exp'd
                    for (g, j, w, qc) in batch:
                        remaining[g] -= 1
                        if remaining[g] == 0:
                            for tcol in (colof[(g, 2 * g)],
                                         colof[(g, 2 * g + 1)]):
                                nc.vector.tensor_mul(
                                    etile[:, tcol:tcol + P],
                                    etile[:, tcol:tcol + P], mtile[:])
                            if last_head and g == NG // 2:
                                # tiles 8..15 done early under big->small
                                # order: store them as soon as ready
                                store = [(nc.gpsimd, out[u, hl][:, NG:, :],
                                          osb[:, NG:, :])]
                            elif last_head and g == 2:
                                store = [(nc.gpsimd, out[u, hl][:, 4:NG, :],
                                          osb[:, 4:NG, :])]
                            elif g == gs[-1]:
                                if last_head:
                                    # only tiles 0..3 remain: 3-way split by
                                    # partitions across SP/Pool/ACT (all idle
                                    # at the kernel tail) to shrink the tail
                                    store = [
                                        (nc.sync, out[u, hl][:43, :4, :],
                                         osb[:43, :4, :]),
                                        (nc.gpsimd, out[u, hl][43:86, :4, :],
                                         osb[43:86, :4, :]),
                                        (nc.scalar, out[u, hl][86:, :4, :],
                                         osb[86:, :4, :]),
                                    ]
                                else:
                                    store = [(nc.gpsimd, out[u, hl], osb[:])]
                            else:
                                store = None
                            pend.append((vtile, etile, colof, osb, g, store))
                            # bound pend loosely here; the between-batches
                            # flush (>= PIPE) does the steady-state draining.
                            # A tight bound would emit the final PV groups of
                            # a head BEFORE the next head's first QK batch,
                            # starving ScalarE across the boundary.
                            while len(pend) > 4:
                                flush_pend()

                batch, bcols = [], 0
                for ch in chunks:
                    if bcols + ch[2] > BCOLS:
                        do_batch(batch, bcols)
                        batch, bcols = [], 0
                        while len(pend) >= PIPE:
                            flush_pend()
                    batch.append(ch)
                    bcols += ch[2]
                if batch:
                    do_batch(batch, bcols)
        while pend:
            flush_pend()

    nc.compile()
    return nc


def _prep_inputs(q, k, v):
    """Host-side sharding/layout. Returns in_maps for the 8 cores."""
    q = np.ascontiguousarray(np.asarray(q, dtype=np.float32))
    k = np.ascontiguousarray(np.asarray(k, dtype=np.float32))
    v = np.ascontiguousarray(np.asarray(v, dtype=np.float32))

    # [B, H(K), D, S] transposed views, contiguous
    qt_all = np.ascontiguousarray(q.reshape(B, S, H, D).transpose(0, 2, 3, 1))
    kt_all = np.ascontiguousarray(k.reshape(B, S, HK, D).transpose(0, 2, 3, 1))
    v4 = v.reshape(B, S, HK, D)

    # tri mask for scoresT diagonal blocks: [r, c] = 1 if c >= r
    r = np.arange(P)[:, None]
    c = np.arange(P)[None, :]
    masks = (c >= r).astype(ml_dtypes.bfloat16)

    in_maps = []
    for core in range(NCORES):
        qs = np.empty((NU, NHL, P, S), dtype=np.float32)
        ks = np.empty((NU, P, S), dtype=np.float32)
        vs = np.empty((NU, P, NT, D + 1), dtype=ml_dtypes.bfloat16)
        for ui in range(NU):
            ug = core * NU + ui
            b, kv = divmod(ug, HK)
            qs[ui] = qt_all[b, kv * REP:(kv + 1) * REP]
            ks[ui] = kt_all[b, kv]
            # V' chunks: [sk_in_chunk, chunk, d] with ones in column D
            vu = v4[b, :, kv, :].reshape(NT, P, D).transpose(1, 0, 2)
            vs[ui, :, :, :D] = vu.astype(ml_dtypes.bfloat16)
            vs[ui, :, :, D] = 1.0
        in_maps.append({"qT": qs, "kT": ks, "vp": vs, "masks": masks})
    return in_maps


def _assemble(results):
    res = np.empty((B, S, H, D), dtype=np.float32)
    for core in range(NCORES):
        arr = results[core]["out"]  # [NU, NHL, P(sq), NT, D]
        for ui in range(NU):
            ug = core * NU + ui
            b, kv = divmod(ug, HK)
            # [NHL, P, NT, D] -> [(NT P) = S, NHL, D]
            blk = arr[ui].transpose(2, 1, 0, 3).reshape(S, NHL, D)
            res[b, :, kv * REP:(kv + 1) * REP, :] = blk
    return res.reshape(B * S, H * D)


def kernel(q, k, v, seq_lens=None, **_unused):
    key = "prog"
    if key not in _CACHE:
        _CACHE[key] = _build_program(qk_f32r=True)
    nc = _CACHE[key]
    in_maps = _prep_inputs(q, k, v)
    res = run_bass_kernel_spmd(nc, in_maps, list(range(NCORES)))
    return _assemble(res.results)



# revision 40
# speedup vs baseline: 1.0975x; 1.0975x over previous
"""Causal GQA attention (B=4, S=2048, H=16, HK=4, D=128) on 8 trn2 cores.

Sharding: 16 (request, kv-head) units, 2 per core. Each unit owns 4 query
heads that share one K/V head (GQA group).

Per-core kernel (per head):
  - scores are computed TRANSPOSED: S_T[sk, sq] = K_chunk^T-stationary
    matmul with Q^T moving (f32r, free dim 256 => full PE rate), so no
    P-transposes are ever needed.
  - exp on ScalarE straight out of PSUM with the 1/sqrt(D) scale folded in,
    output bf16 to SBUF.
  - causal masking is multiplicative on the two diagonal chunks per group.
  - PV: lhsT = expT chunk [sk,128sq] (bf16 stationary), rhs = V' chunk
    [sk, 129] where column 128 is ones => row-sums accumulate for free in
    PSUM column 128, and the output lands already [sq, d].
  - normalize with DVE reciprocal + tensor_scalar_mul, DMA out.
"""

import sys

if "/opt/trn_rl_repo" not in sys.path:
    sys.path.insert(0, "/opt/trn_rl_repo")

from contextlib import ExitStack

import ml_dtypes
import numpy as np

import concourse.bass as bass  # noqa: F401  (registers engine classes)
import concourse.tile as tile
from concourse import bacc, mybir
from concourse.bass_utils import run_bass_kernel_spmd

B = 4
S = 2048
H = 16
HK = 4
D = 128
REP = H // HK          # query heads per kv head
SCALE = float(1.0 / np.float32(np.sqrt(D)))

NCORES = 8
NU = 2                 # units (request, kv-head) per core
NHL = REP              # query heads per unit
P = 128
NT = S // P            # 16 sk/sq tiles per sequence
SQG = 256              # sq columns per group (>=256 keeps f32r at full rate)
NG = S // SQG          # 8 groups per head
GB = 6                 # score chunks per PSUM batch ([128, 6*256] = 3 banks)

F32 = mybir.dt.float32
F32R = mybir.dt.float32r
BF16 = mybir.dt.bfloat16

_CACHE = {}


def _build_program(qk_f32r: bool = False, pipe: int = 3, split_loads: bool = False):
    nc = bacc.Bacc("TRN2", target_bir_lowering=False, debug=False,
                   num_devices=NCORES)
    QKDT = BF16
    qT = nc.dram_tensor("qT", [NU, NHL, P, S], QKDT, kind="ExternalInput").ap()
    kT = nc.dram_tensor("kT", [NU, P, S], QKDT, kind="ExternalInput").ap()
    vp = nc.dram_tensor("vp", [NU, P, NT, D + 1], BF16, kind="ExternalInput").ap()
    masks = nc.dram_tensor("masks", [P, P], BF16, kind="ExternalInput").ap()
    # [unit, head, sq_partition, q_tile, d] -- one DMA per head, 8KB rows
    out = nc.dram_tensor("out", [NU, NHL, P, NT, D], F32, kind="ExternalOutput").ap()

    with tile.TileContext(nc) as tc, ExitStack() as ctx:
        kpool = ctx.enter_context(tc.tile_pool(name="kT", bufs=2))
        qpool = ctx.enter_context(tc.tile_pool(name="qT", bufs=2))
        vpool = ctx.enter_context(tc.tile_pool(name="vp", bufs=2))
        epool = ctx.enter_context(tc.tile_pool(name="expT", bufs=3))
        mpool = ctx.enter_context(tc.tile_pool(name="masks", bufs=1))
        opool = ctx.enter_context(tc.tile_pool(name="osb", bufs=2))
        rpool = ctx.enter_context(tc.tile_pool(name="recip", bufs=4))
        spsum = ctx.enter_context(tc.tile_pool(name="scores", bufs=2, space="PSUM"))
        ppsum = ctx.enter_context(tc.tile_pool(name="pv", bufs=2, space="PSUM"))

        mtile = mpool.tile([P, P], BF16)

        # PE HAM warm-up: dummy matmuls on scratch SBUF during the initial
        # K/Q DMA wait so the first real QK batch runs above the low pstate.
        # Sized to END roughly when the first K/Q pieces land (~2.1us).
        wpool = ctx.enter_context(tc.tile_pool(name="warm", bufs=1))
        wtile = wpool.tile([P, 512], BF16)
        nc.vector.memset(wtile[:].bitcast(mybir.dt.uint16), 0)
        for _ in range(4):
            wpsum = ppsum.tile([P, D + 1], F32, tag="pv", name="wpsum")
            nc.tensor.matmul(wpsum[:], lhsT=wtile[:, :P],
                             rhs=wtile[:, :D + 1], start=True, stop=True)

        HEADCOLS = sum((2 * g + 1) * SQG + P for g in range(NG))  # 17408
        BCOLS = GB * SQG                 # psum batch capacity (3 banks)

        # Software pipeline: PV work is queued as individual matmul pieces
        # and drained a few at a time between QK/exp batches. PE is in-order,
        # so keeping each injected PV block small (KPV matmuls) guarantees
        # the next QK batch -- and with it ScalarE's next exp -- is never
        # parked behind a long PV run.
        pvwork = []        # list of zero-arg closures, each emits 1 PE op
        KPV = 14           # max PV matmuls injected per batch boundary

        def queue_pv(vtile, etile, colof, osb, g, store, pool, act_norm=False):
            """Queue PV + normalize pieces for the two q-tiles of group g."""
            state = {}

            def start_tile(cc):
                t = 2 * g + cc
                if pool is None or cc == 1:
                    ptile = ppsum.tile([P, D + 1], F32, tag="pv", name="ptile")
                else:
                    # tail PVs rotate through the (by then idle) scores
                    # banks so the last four PV tiles land in four distinct
                    # banks and never wait on a normalize read
                    ptile = pool.tile([P, BCOLS], F32, tag="scores",
                                      name="tailpv")[:, :D + 1]
                state[cc] = ptile
                return ptile, t

            def mm(cc, j):
                def run():
                    if j == 0:
                        ptile, t = start_tile(cc)
                    else:
                        ptile, t = state[cc], 2 * g + cc
                    c0 = colof[(g, j)] + (0 if (cc == 1 and j == t) else cc * P)
                    nc.tensor.matmul(
                        ptile[:],
                        lhsT=etile[:, c0:c0 + P],
                        rhs=vtile[:, j, :],
                        start=(j == 0), stop=(j == t),
                    )
                    if j == t:
                        rec = rpool.tile([P, 1], F32)
                        nc.vector.reciprocal(rec[:], ptile[:, D:D + 1])
                        if act_norm and cc == 0:
                            # tail tiles: run half the normalizes on the (by
                            # then idle) ACT engine so the final normalize ->
                            # store chain isn't serialized on DVE
                            nc.scalar.activation(
                                osb[:, t, :], ptile[:, 0:D],
                                mybir.ActivationFunctionType.Copy,
                                scale=rec[:])
                        else:
                            nc.vector.tensor_scalar_mul(osb[:, t, :],
                                                        ptile[:, 0:D], rec[:])
                        if cc == 1 and store is not None:
                            for eng, dst, src in store:
                                eng.dma_start(dst, src)
                return run

            for cc in range(2):
                for j in range(2 * g + cc + 1):
                    pvwork.append(mm(cc, j))

        def drain_pv(n):
            for _ in range(min(n, len(pvwork))):
                pvwork.pop(0)()

        for u in range(NU):
            ktile = kpool.tile([P, S], QKDT)
            if u == 0:
                # The cost model serializes DMA transfers in ready-order, so
                # split k into 3 pieces on SP (consumption order) and put the
                # tiny opening-q pieces + masks on the Pool SWDGE queue; the
                # ACT queue stays clear for exp dispatch (its LoadActFuncSet
                # blocks early DMA generation there).
                nc.sync.dma_start(ktile[:, :384], kT[u][:, :384])
                nc.sync.dma_start(ktile[:, 384:1152], kT[u][:, 384:1152])
                nc.sync.dma_start(ktile[:, 1152:], kT[u][:, 1152:])
            else:
                nc.sync.dma_start(ktile[:], kT[u])
            vtile = vpool.tile([P, NT, D + 1], BF16)
            for hl in range(NHL):
                qtile = qpool.tile([P, S], QKDT)
                if u == 0 and hl == 0:
                    # groups run big->small and the first batch is 3 chunks:
                    # the opening batches only read q columns [1792:2048]
                    nc.gpsimd.dma_start(qtile[:, 1792:], qT[u, hl][:, 1792:])
                    nc.gpsimd.dma_start(qtile[:, 1536:1792],
                                        qT[u, hl][:, 1536:1792])
                    nc.sync.dma_start(qtile[:, :1536], qT[u, hl][:, :1536])
                    nc.gpsimd.dma_start(mtile[:], masks[:])
                else:
                    nc.sync.dma_start(qtile[:], qT[u, hl])
                if hl == 0:
                    nc.sync.dma_start(vtile[:], vp[u])
                osb = opool.tile([P, NT, D], F32)
                etile = epool.tile([P, HEADCOLS], BF16)
                last_head = (u == NU - 1 and hl == NHL - 1)
                # groups big->small: every head ends on tiny PV work and the
                # next head opens with big QK batches, keeping ScalarE fed
                # across head boundaries (and the kernel tail short)
                gs = list(range(NG - 1, -1, -1))

                # chunk stream for this head: full 256-wide chunks of a PAIR
                # of groups, then their two 128-wide diagonal half-chunks
                # back-to-back. Keeps every 256-wide PSUM write 256-aligned so
                # no matmul output crosses a PSUM bank boundary.
                chunks = []
                for ga, gb in zip(gs[0::2], gs[1::2]):
                    for j in range(2 * ga + 1):
                        chunks.append((ga, j, SQG, ga * SQG))
                    chunks.append((ga, 2 * ga + 1, P, ga * SQG + P))
                    chunks.append((gb, 2 * gb + 1, P, gb * SQG + P))
                    for j in range(2 * gb + 1):
                        chunks.append((gb, j, SQG, gb * SQG))
                remaining = {g: 2 * g + 2 for g in gs}
                colof = {}
                acc = 0
                for (g, j, w, qc) in chunks:
                    colof[(g, j)] = acc
                    acc += w

                def do_batch(batch, bcols):
                    stile = spsum.tile([P, BCOLS], F32, tag="scores")
                    ncols = 0
                    for (g, j, w, qc) in batch:
                        nc.tensor.matmul(
                            stile[:, ncols:ncols + w],
                            lhsT=ktile[:, j * P:(j + 1) * P],
                            rhs=qtile[:, qc:qc + w],
                            start=True, stop=True,
                        )
                        ncols += w
                    e0 = colof[(batch[0][0], batch[0][1])]
                    nc.scalar.activation(
                        etile[:, e0:e0 + ncols],
                        stile[:, :ncols],
                        mybir.ActivationFunctionType.Exp,
                        scale=SCALE,
                    )
                    # a group is complete once all its chunks are exp'd.
                    # Groups complete big->small (descending) which keeps the
                    # steady-state pipeline tight; only the LAST head's final
                    # batch (groups 1+0 complete together) queues ascending,
                    # so g0's normalize+store starts ~1us earlier at the tail.
                    for (g, j, w, qc) in batch:
                        remaining[g] -= 1
                    done = [g for g in dict.fromkeys(c[0] for c in batch)
                            if remaining[g] == 0]
                    if last_head and set(done) == {0, 1}:
                        # only the very last batch flips to ascending: stores
                        # are per-group there, so g0's chain can lead. Other
                        # multi-group batches must stay descending -- e.g.
                        # g2's store reads g3's osb tiles and therefore has
                        # to queue after g3's normalizes.
                        done = sorted(done)
                    for g in done:
                        if True:
                            for tcol in (colof[(g, 2 * g)],
                                         colof[(g, 2 * g + 1)]):
                                nc.vector.tensor_mul(
                                    etile[:, tcol:tcol + P],
                                    etile[:, tcol:tcol + P], mtile[:])
                            if last_head and g == NG // 2:
                                # tiles 8..15 done early under big->small
                                # order: store them as soon as ready
                                store = [(nc.gpsimd, out[u, hl][:, NG:, :],
                                          osb[:, NG:, :])]
                            elif last_head and g == 2:
                                store = [(nc.sync, out[u, hl][:, 4:6, :],
                                          osb[:, 4:6, :]),
                                         (nc.gpsimd, out[u, hl][:, 6:NG, :],
                                          osb[:, 6:NG, :])]
                            elif last_head and g == 1:
                                # g1 is queued after g0 (ascending), so this
                                # is the final store of the kernel
                                store = [(nc.scalar, out[u, hl][:, 2:4, :],
                                          osb[:, 2:4, :])]
                            elif last_head and g == 0:
                                store = [(nc.sync, out[u, hl][:, 0:2, :],
                                          osb[:, 0:2, :])]
                            elif g == gs[-1]:
                                # descending order: g0 is queued last, so the
                                # whole-head store (which reads every osb
                                # tile) rides on its final normalize
                                store = [(nc.gpsimd, out[u, hl], osb[:])]
                            else:
                                store = None
                            # the last head's final two PV groups run after
                            # the last exp: pull their PSUM from the (by then
                            # idle) scores pool so they don't serialize on the
                            # two pv banks behind group 2's normalize.
                            pvpool = spsum if (last_head and g <= 1) else None
                            queue_pv(vtile, etile, colof, osb, g, store,
                                     pvpool,
                                     act_norm=(last_head and g <= 1))

                batch, bcols = [], 0
                nbatch = 0
                for ch in chunks:
                    # the first batch of each head is kept small (3 chunks) so
                    # the next head's opening QK+exp slots in right behind the
                    # previous head's final exp instead of stalling ScalarE.
                    # On the last head the final batch is split at the g1/g0
                    # boundary so group 1 completes (and its PV+normalize
                    # chain starts) while group 0's exp still runs.
                    cap = 768 if nbatch == 0 else BCOLS
                    split = (last_head and ch[0] == 0 and batch
                             and batch[-1][0] == 1)
                    if bcols + ch[2] > cap or split:
                        do_batch(batch, bcols)
                        nbatch += 1
                        batch, bcols = [], 0
                        drain_pv(KPV)
                    batch.append(ch)
                    bcols += ch[2]
                if batch:
                    do_batch(batch, bcols)
        drain_pv(len(pvwork))

    nc.compile()
    return nc


def _prep_inputs(q, k, v):
    """Host-side sharding/layout. Returns in_maps for the 8 cores."""
    q = np.ascontiguousarray(np.asarray(q, dtype=np.float32))
    k = np.ascontiguousarray(np.asarray(k, dtype=np.float32))
    v = np.ascontiguousarray(np.asarray(v, dtype=np.float32))

    # [B, H(K), D, S] transposed views, contiguous
    qt_all = np.ascontiguousarray(q.reshape(B, S, H, D).transpose(0, 2, 3, 1))
    kt_all = np.ascontiguousarray(k.reshape(B, S, HK, D).transpose(0, 2, 3, 1))
    v4 = v.reshape(B, S, HK, D)

    # tri mask for scoresT diagonal blocks: [r, c] = 1 if c >= r
    r = np.arange(P)[:, None]
    c = np.arange(P)[None, :]
    masks = (c >= r).astype(ml_dtypes.bfloat16)

    in_maps = []
    for core in range(NCORES):
        qs = np.empty((NU, NHL, P, S), dtype=ml_dtypes.bfloat16)
        ks = np.empty((NU, P, S), dtype=ml_dtypes.bfloat16)
        vs = np.empty((NU, P, NT, D + 1), dtype=ml_dtypes.bfloat16)
        for ui in range(NU):
            ug = core * NU + ui
            b, kv = divmod(ug, HK)
            qs[ui] = qt_all[b, kv * REP:(kv + 1) * REP]
            ks[ui] = kt_all[b, kv]
            # V' chunks: [sk_in_chunk, chunk, d] with ones in column D
            vu = v4[b, :, kv, :].reshape(NT, P, D).transpose(1, 0, 2)
            vs[ui, :, :, :D] = vu.astype(ml_dtypes.bfloat16)
            vs[ui, :, :, D] = 1.0
        in_maps.append({"qT": qs, "kT": ks, "vp": vs, "masks": masks})
    return in_maps


def _assemble(results):
    res = np.empty((B, S, H, D), dtype=np.float32)
    for core in range(NCORES):
        arr = results[core]["out"]  # [NU, NHL, P(sq), NT, D]
        for ui in range(NU):
            ug = core * NU + ui
            b, kv = divmod(ug, HK)
            # [NHL, P, NT, D] -> [(NT P) = S, NHL, D]
            blk = arr[ui].transpose(2, 1, 0, 3).reshape(S, NHL, D)
            res[b, :, kv * REP:(kv + 1) * REP, :] = blk
    return res.reshape(B * S, H * D)


def kernel(q, k, v, seq_lens=None, **_unused):
    key = "prog"
    if key not in _CACHE:
        _CACHE[key] = _build_program()
    nc = _CACHE[key]
    in_maps = _prep_inputs(q, k, v)
    res = run_bass_kernel_spmd(nc, in_maps, list(range(NCORES)))
    return _assemble(res.results)



# revision 57
# speedup vs baseline: 1.1086x; 1.0101x over previous
"""Causal GQA attention (B=4, S=2048, H=16, HK=4, D=128) on 8 trn2 cores.

Sharding: 16 (request, kv-head) units, 2 per core. Each unit owns 4 query
heads that share one K/V head (GQA group).

Per-core kernel (per head):
  - scores are computed TRANSPOSED: S_T[sk, sq] = K_chunk^T-stationary
    matmul with Q^T moving (f32r, free dim 256 => full PE rate), so no
    P-transposes are ever needed.
  - exp on ScalarE straight out of PSUM with the 1/sqrt(D) scale folded in,
    output bf16 to SBUF.
  - causal masking is multiplicative on the two diagonal chunks per group.
  - PV: lhsT = expT chunk [sk,128sq] (bf16 stationary), rhs = V' chunk
    [sk, 129] where column 128 is ones => row-sums accumulate for free in
    PSUM column 128, and the output lands already [sq, d].
  - normalize with DVE reciprocal + tensor_scalar_mul, DMA out.
"""

import sys

if "/opt/trn_rl_repo" not in sys.path:
    sys.path.insert(0, "/opt/trn_rl_repo")

from contextlib import ExitStack

import ml_dtypes
import numpy as np

import concourse.bass as bass  # noqa: F401  (registers engine classes)
import concourse.tile as tile
from concourse import bacc, mybir
from concourse.bass_utils import run_bass_kernel_spmd

B = 4
S = 2048
H = 16
HK = 4
D = 128
REP = H // HK          # query heads per kv head
SCALE = float(1.0 / np.float32(np.sqrt(D)))

NCORES = 8
NU = 2                 # units (request, kv-head) per core
NHL = REP              # query heads per unit
P = 128
NT = S // P            # 16 sk/sq tiles per sequence
SQG = 256              # sq columns per group (>=256 keeps f32r at full rate)
NG = S // SQG          # 8 groups per head
GB = 6                 # score chunks per PSUM batch ([128, 6*256] = 3 banks)

F32 = mybir.dt.float32
F32R = mybir.dt.float32r
BF16 = mybir.dt.bfloat16

_CACHE = {}


def _build_program(qk_f32r: bool = False, pipe: int = 3, split_loads: bool = False):
    nc = bacc.Bacc("TRN2", target_bir_lowering=False, debug=False,
                   num_devices=NCORES)
    QKDT = BF16
    qT = nc.dram_tensor("qT", [NU, NHL, P, S], QKDT, kind="ExternalInput").ap()
    kT = nc.dram_tensor("kT", [NU, P, S], QKDT, kind="ExternalInput").ap()
    vp = nc.dram_tensor("vp", [NU, P, NT, D + 1], BF16, kind="ExternalInput").ap()
    masks = nc.dram_tensor("masks", [P, P], BF16, kind="ExternalInput").ap()
    # [unit, head, sq_partition, q_tile, d] -- one DMA per head, 8KB rows
    out = nc.dram_tensor("out", [NU, NHL, P, NT, D], F32, kind="ExternalOutput").ap()

    with tile.TileContext(nc) as tc, ExitStack() as ctx:
        kpool = ctx.enter_context(tc.tile_pool(name="kT", bufs=2))
        qpool = ctx.enter_context(tc.tile_pool(name="qT", bufs=2))
        vpool = ctx.enter_context(tc.tile_pool(name="vp", bufs=2))
        epool = ctx.enter_context(tc.tile_pool(name="expT", bufs=3))
        mpool = ctx.enter_context(tc.tile_pool(name="masks", bufs=1))
        opool = ctx.enter_context(tc.tile_pool(name="osb", bufs=2))
        rpool = ctx.enter_context(tc.tile_pool(name="recip", bufs=4))
        spsum = ctx.enter_context(tc.tile_pool(name="scores", bufs=2, space="PSUM"))
        ppsum = ctx.enter_context(tc.tile_pool(name="pv", bufs=2, space="PSUM"))

        mtile = mpool.tile([P, P], BF16)

        # PE HAM warm-up: dummy matmuls on scratch SBUF during the initial
        # K/Q DMA wait so the first real QK batch runs above the low pstate.
        # Sized to END roughly when the first K/Q pieces land (~2.1us).
        wpool = ctx.enter_context(tc.tile_pool(name="warm", bufs=1))
        wtile = wpool.tile([P, 512], BF16)
        nc.vector.memset(wtile[:].bitcast(mybir.dt.uint16), 0)
        for _ in range(4):
            wpsum = ppsum.tile([P, D + 1], F32, tag="pv", name="wpsum")
            nc.tensor.matmul(wpsum[:], lhsT=wtile[:, :P],
                             rhs=wtile[:, :D + 1], start=True, stop=True)

        HEADCOLS = sum((2 * g + 1) * SQG + P for g in range(NG))  # 17408
        BCOLS = GB * SQG                 # psum batch capacity (3 banks)

        # Software pipeline: PV work is queued as individual matmul pieces
        # and drained a few at a time between QK/exp batches. PE is in-order,
        # so keeping each injected PV block small (KPV matmuls) guarantees
        # the next QK batch -- and with it ScalarE's next exp -- is never
        # parked behind a long PV run.
        pvwork = []        # list of zero-arg closures, each emits 1 PE op
        KPV = 10           # max PV matmuls injected per batch boundary

        def queue_pv(vtile, etile, colof, osb, g, store, pool, act_norm=False):
            """Queue PV + normalize pieces for the two q-tiles of group g."""
            state = {}

            def start_tile(cc):
                t = 2 * g + cc
                if pool is None or cc == 1:
                    ptile = ppsum.tile([P, D + 1], F32, tag="pv", name="ptile")
                else:
                    # tail PVs rotate through the (by then idle) scores
                    # banks so the last four PV tiles land in four distinct
                    # banks and never wait on a normalize read
                    ptile = pool.tile([P, BCOLS], F32, tag="scores",
                                      name="tailpv")[:, :D + 1]
                state[cc] = ptile
                return ptile, t

            def mm(cc, j):
                def run():
                    if j == 0:
                        ptile, t = start_tile(cc)
                    else:
                        ptile, t = state[cc], 2 * g + cc
                    c0 = colof[(g, j)] + (0 if (cc == 1 and j == t) else cc * P)
                    nc.tensor.matmul(
                        ptile[:],
                        lhsT=etile[:, c0:c0 + P],
                        rhs=vtile[:, j, :],
                        start=(j == 0), stop=(j == t),
                    )
                    if j == t:
                        rec = rpool.tile([P, 1], F32)
                        nc.vector.reciprocal(rec[:], ptile[:, D:D + 1])
                        if act_norm and cc == 0:
                            # tail tiles: run half the normalizes on the (by
                            # then idle) ACT engine so the final normalize ->
                            # store chain isn't serialized on DVE
                            nc.scalar.activation(
                                osb[:, t, :], ptile[:, 0:D],
                                mybir.ActivationFunctionType.Copy,
                                scale=rec[:])
                        else:
                            nc.vector.tensor_scalar_mul(osb[:, t, :],
                                                        ptile[:, 0:D], rec[:])
                        # store: list fires after cc==1; dict fires per-cc
                        if isinstance(store, dict):
                            for eng, dst, src in store.get(cc, ()):
                                eng.dma_start(dst, src)
                        elif cc == 1 and store is not None:
                            for eng, dst, src in store:
                                eng.dma_start(dst, src)
                return run

            for cc in range(2):
                for j in range(2 * g + cc + 1):
                    pvwork.append(mm(cc, j))

        def drain_pv(n):
            for _ in range(min(n, len(pvwork))):
                pvwork.pop(0)()

        for u in range(NU):
            ktile = kpool.tile([P, S], QKDT)
            if u == 0:
                # The cost model serializes DMA transfers in ready-order, so
                # split k into 3 pieces on SP (consumption order) and put the
                # tiny opening-q pieces + masks on the Pool SWDGE queue; the
                # ACT queue stays clear for exp dispatch (its LoadActFuncSet
                # blocks early DMA generation there).
                nc.sync.dma_start(ktile[:, :384], kT[u][:, :384])
                nc.sync.dma_start(ktile[:, 384:1152], kT[u][:, 384:1152])
                nc.sync.dma_start(ktile[:, 1152:], kT[u][:, 1152:])
            else:
                nc.sync.dma_start(ktile[:], kT[u])
            vtile = vpool.tile([P, NT, D + 1], BF16)
            for hl in range(NHL):
                qtile = qpool.tile([P, S], QKDT)
                if u == 0 and hl == 0:
                    # groups run big->small and the first batch is 3 chunks:
                    # the opening batches only read q columns [1792:2048]
                    nc.gpsimd.dma_start(qtile[:, 1792:], qT[u, hl][:, 1792:])
                    nc.gpsimd.dma_start(qtile[:, 1536:1792],
                                        qT[u, hl][:, 1536:1792])
                    nc.sync.dma_start(vtile[:], vp[u])
                    nc.sync.dma_start(qtile[:, :1536], qT[u, hl][:, :1536])
                    nc.gpsimd.dma_start(mtile[:], masks[:])
                elif hl == 0:
                    nc.sync.dma_start(qtile[:], qT[u, hl])
                    nc.sync.dma_start(vtile[:], vp[u])
                else:
                    nc.sync.dma_start(qtile[:], qT[u, hl])
                osb = opool.tile([P, NT, D], F32)
                etile = epool.tile([P, HEADCOLS], BF16)
                last_head = (u == NU - 1 and hl == NHL - 1)
                # groups big->small: every head ends on tiny PV work and the
                # next head opens with big QK batches, keeping ScalarE fed
                # across head boundaries (and the kernel tail short)
                gs = list(range(NG - 1, -1, -1))

                # chunk stream for this head: full 256-wide chunks of a PAIR
                # of groups, then their two 128-wide diagonal half-chunks
                # back-to-back. Keeps every 256-wide PSUM write 256-aligned so
                # no matmul output crosses a PSUM bank boundary.
                chunks = []
                for ga, gb in zip(gs[0::2], gs[1::2]):
                    for j in range(2 * ga + 1):
                        chunks.append((ga, j, SQG, ga * SQG))
                    if last_head and gb == 0:
                        # final pair of the kernel: finish g0 first and end
                        # on g1's single 128-wide diagonal chunk, so the very
                        # last exp is tiny and both tail PV chains overlap
                        # the preceding exps
                        chunks.append((gb, 2 * gb + 1, P, gb * SQG + P))
                        for j in range(2 * gb + 1):
                            chunks.append((gb, j, SQG, gb * SQG))
                        chunks.append((ga, 2 * ga + 1, P, ga * SQG + P))
                    else:
                        chunks.append((ga, 2 * ga + 1, P, ga * SQG + P))
                        chunks.append((gb, 2 * gb + 1, P, gb * SQG + P))
                        for j in range(2 * gb + 1):
                            chunks.append((gb, j, SQG, gb * SQG))
                remaining = {g: 2 * g + 2 for g in gs}
                colof = {}
                acc = 0
                for (g, j, w, qc) in chunks:
                    colof[(g, j)] = acc
                    acc += w

                def do_batch(batch, bcols):
                    stile = spsum.tile([P, BCOLS], F32, tag="scores")
                    ncols = 0
                    for (g, j, w, qc) in batch:
                        nc.tensor.matmul(
                            stile[:, ncols:ncols + w],
                            lhsT=ktile[:, j * P:(j + 1) * P],
                            rhs=qtile[:, qc:qc + w],
                            start=True, stop=True,
                        )
                        ncols += w
                    e0 = colof[(batch[0][0], batch[0][1])]
                    nc.scalar.activation(
                        etile[:, e0:e0 + ncols],
                        stile[:, :ncols],
                        mybir.ActivationFunctionType.Exp,
                        scale=SCALE,
                    )
                    # a group is complete once all its chunks are exp'd.
                    # Groups complete big->small (descending) which keeps the
                    # steady-state pipeline tight; only the LAST head's final
                    # batch (groups 1+0 complete together) queues ascending,
                    # so g0's normalize+store starts ~1us earlier at the tail.
                    for (g, j, w, qc) in batch:
                        remaining[g] -= 1
                    done = [g for g in dict.fromkeys(c[0] for c in batch)
                            if remaining[g] == 0]
                    if last_head and set(done) == {0, 1}:
                        # only the very last batch flips to ascending: stores
                        # are per-group there, so g0's chain can lead. Other
                        # multi-group batches must stay descending -- e.g.
                        # g2's store reads g3's osb tiles and therefore has
                        # to queue after g3's normalizes.
                        done = sorted(done)
                    for g in done:
                        if True:
                            for tcol in (colof[(g, 2 * g)],
                                         colof[(g, 2 * g + 1)]):
                                nc.vector.tensor_mul(
                                    etile[:, tcol:tcol + P],
                                    etile[:, tcol:tcol + P], mtile[:])
                            if last_head and g == NG // 2:
                                # tiles 8..15 done early under big->small
                                # order: store them as soon as ready
                                store = [(nc.gpsimd, out[u, hl][:, NG:, :],
                                          osb[:, NG:, :])]
                            elif last_head and g == 2:
                                store = [(nc.sync, out[u, hl][:, 4:6, :],
                                          osb[:, 4:6, :]),
                                         (nc.gpsimd, out[u, hl][:, 6:NG, :],
                                          osb[:, 6:NG, :])]
                            elif last_head and g == 1:
                                # g1 is queued after g0 (ascending), so this
                                # is the final store of the kernel
                                store = [(nc.scalar, out[u, hl][:, 2:4, :],
                                          osb[:, 2:4, :])]
                            elif last_head and g == 0:
                                store = [(nc.sync, out[u, hl][:, 0:2, :],
                                          osb[:, 0:2, :])]
                            elif g == gs[-1]:
                                # descending order: g0 is queued last, so the
                                # whole-head store (which reads every osb
                                # tile) rides on its final normalize
                                store = [(nc.gpsimd, out[u, hl], osb[:])]
                            else:
                                store = None
                            # the last head's final two PV groups run after
                            # the last exp: pull their PSUM from the (by then
                            # idle) scores pool so they don't serialize on the
                            # two pv banks behind group 2's normalize.
                            pvpool = spsum if (last_head and g <= 1) else None
                            queue_pv(vtile, etile, colof, osb, g, store,
                                     pvpool,
                                     act_norm=(last_head and g <= 1))

                batch, bcols = [], 0
                nbatch = 0
                for ch in chunks:
                    # the first batch of each head is kept small (3 chunks) so
                    # the next head's opening QK+exp slots in right behind the
                    # previous head's final exp instead of stalling ScalarE.
                    # On the last head the final batch is split at the g1/g0
                    # boundary so group 1 completes (and its PV+normalize
                    # chain starts) while group 0's exp still runs.
                    cap = 768 if nbatch == 0 else BCOLS
                    if bcols + ch[2] > cap:
                        do_batch(batch, bcols)
                        nbatch += 1
                        batch, bcols = [], 0
                        # on the last head drain harder so only the final two
                        # groups' PV work remains after the last exp
                        drain_pv(KPV + (8 if last_head else 0))
                    batch.append(ch)
                    bcols += ch[2]
                if batch:
                    do_batch(batch, bcols)
        drain_pv(len(pvwork))

    nc.compile()
    return nc


def _prep_inputs(q, k, v):
    """Host-side sharding/layout. Returns in_maps for the 8 cores."""
    q = np.ascontiguousarray(np.asarray(q, dtype=np.float32))
    k = np.ascontiguousarray(np.asarray(k, dtype=np.float32))
    v = np.ascontiguousarray(np.asarray(v, dtype=np.float32))

    # [B, H(K), D, S] transposed views, contiguous
    qt_all = np.ascontiguousarray(q.reshape(B, S, H, D).transpose(0, 2, 3, 1))
    kt_all = np.ascontiguousarray(k.reshape(B, S, HK, D).transpose(0, 2, 3, 1))
    v4 = v.reshape(B, S, HK, D)

    # tri mask for scoresT diagonal blocks: [r, c] = 1 if c >= r
    r = np.arange(P)[:, None]
    c = np.arange(P)[None, :]
    masks = (c >= r).astype(ml_dtypes.bfloat16)

    in_maps = []
    for core in range(NCORES):
        qs = np.empty((NU, NHL, P, S), dtype=ml_dtypes.bfloat16)
        ks = np.empty((NU, P, S), dtype=ml_dtypes.bfloat16)
        vs = np.empty((NU, P, NT, D + 1), dtype=ml_dtypes.bfloat16)
        for ui in range(NU):
            ug = core * NU + ui
            b, kv = divmod(ug, HK)
            qs[ui] = qt_all[b, kv * REP:(kv + 1) * REP]
            ks[ui] = kt_all[b, kv]
            # V' chunks: [sk_in_chunk, chunk, d] with ones in column D
            vu = v4[b, :, kv, :].reshape(NT, P, D).transpose(1, 0, 2)
            vs[ui, :, :, :D] = vu.astype(ml_dtypes.bfloat16)
            vs[ui, :, :, D] = 1.0
        in_maps.append({"qT": qs, "kT": ks, "vp": vs, "masks": masks})
    return in_maps


def _assemble(results):
    res = np.empty((B, S, H, D), dtype=np.float32)
    for core in range(NCORES):
        arr = results[core]["out"]  # [NU, NHL, P(sq), NT, D]
        for ui in range(NU):
            ug = core * NU + ui
            b, kv = divmod(ug, HK)
            # [NHL, P, NT, D] -> [(NT P) = S, NHL, D]
            blk = arr[ui].transpose(2, 1, 0, 3).reshape(S, NHL, D)
            res[b, :, kv * REP:(kv + 1) * REP, :] = blk
    return res.reshape(B * S, H * D)


def kernel(q, k, v, seq_lens=None, **_unused):
    key = "prog"
    if key not in _CACHE:
        _CACHE[key] = _build_program()
    nc = _CACHE[key]
    in_maps = _prep_inputs(q, k, v)
    res = run_bass_kernel_spmd(nc, in_maps, list(range(NCORES)))
    return _assemble(res.results)

